# revision 1
# baseline (speedup 1.0000x reference)
"""GRU layer kernel for Trainium2, data-parallel over 8 NeuronCores.

Strategy (feature-major / weight-stationary):
  - Shard batch N=2048 -> 8 cores x NB=256.
  - On host: transpose inputs to feature-major xT[t] = [D, NB] per core, and
    pre-pack each weight matrix W[dout,din] into the PE lhsT tile layout
    (tile (k,m): lhsT[p, q] = W[m*128+q, k*128+p]).
  - On chip per timestep (all matmuls fp32r, PSUM fp32 accumulate):
      z_pre.T = Wz_x| x_t.T  +  Wz_h| h.T      (8 K-tiles into PSUM)
      r_pre.T = likewise
      g_pre.T = Wh_x| x_t.T  +  Wh_h| (r*h).T
      z,r = sigmoid(+bias) on ACT; g = tanh(+bias); blend on DVE.
    x-projection matmuls of step t+1 have no dependency on step t and fill
    the PE pipeline while ACT/DVE resolve the recurrence.
  - Output written feature-major [T, D, NB] per core; host transposes back.
"""
import os
import numpy as np

N, D = 2048, 512
T = int(os.environ.get("GRU_T", "64"))
NC = 8
NB = N // NC          # 256 batch rows per core
KT = D // 128         # 4 k-tiles
MT = D // 128         # 4 m-tiles

MM_DT = os.environ.get("GRU_MM_DT", "fp32r")   # fp32r | fp32

_CACHE = {}
LAST_RESULT = None


def _build_nc():
    import concourse.bacc as bacc
    import concourse.mybir as mybir
    from concourse.tile import TileContext

    f32 = mybir.dt.float32
    # fp32r: full-speed PE path (1 cyc/row vs 4 for fp32). Requires every
    # matmul operand to be produced as float32r (DRAM decl or on-chip
    # producer op writes an f32r-typed tile).
    mdt = mybir.dt.float32r if MM_DT == "fp32r" else f32
    Sig = mybir.ActivationFunctionType.Sigmoid
    Tanh = mybir.ActivationFunctionType.Tanh

    nc = bacc.Bacc("TRN2", target_bir_lowering=False, debug=False, num_devices=NC)

    xt_d = nc.dram_tensor("xt", [T, D, NB], mdt, kind="ExternalInput")
    w_d = {}
    for wname in ("wzx", "wzh", "wrx", "wrh", "whx", "whh"):
        w_d[wname] = nc.dram_tensor(wname, [128, KT * MT * 128], mdt, kind="ExternalInput")
    b_d = {}
    for bname in ("bz", "br", "bh"):
        b_d[bname] = nc.dram_tensor(bname, [128, MT], f32, kind="ExternalInput")
    out_d = nc.dram_tensor("out", [T, D, NB], f32, kind="ExternalOutput")

    with TileContext(nc) as tc:
        with (
            tc.tile_pool(name="wsb", bufs=1) as wsb,
            tc.tile_pool(name="xsb", bufs=4) as xsb,
            tc.tile_pool(name="ssb", bufs=2) as ssb,
            tc.tile_pool(name="hsb", bufs=3) as hsb,
            tc.tile_pool(name="psum", bufs=1, space="PSUM") as psum,
        ):
            w_sb = {}
            for wname in w_d:
                w_sb[wname] = wsb.tile([128, KT * MT * 128], mdt, name=f"w_{wname}")
            b_sb = {}
            for bname in b_d:
                b_sb[bname] = wsb.tile([128, MT], f32, name=f"b_{bname}")

            from concourse.tile import add_dep_helper

            def wdma(wname, nchunks=4):
                # chunked so the pieces spread across DMA queues
                insts = []
                cw = KT * MT * 128 // nchunks
                for u in range(nchunks):
                    insts.append(
                        nc.sync.dma_start(out=w_sb[wname][:, u * cw:(u + 1) * cw],
                                          in_=w_d[wname][:, u * cw:(u + 1) * cw]))
                return insts

            # priority set: everything t=0's first matmuls need. All DMA
            # queues round-robin, so the late weight DMAs are gated on the
            # priority set's completion to give it the full HBM bandwidth.
            pri = []
            pri += wdma("wzx")
            pri += wdma("whx")
            pri.append(nc.sync.dma_start(out=b_sb["bz"][:], in_=b_d["bz"][:]))
            pri.append(nc.sync.dma_start(out=b_sb["bh"][:], in_=b_d["bh"][:]))
            xt0 = xsb.tile([128, KT, NB], mdt, name="x0", tag="xt")
            pri.append(nc.sync.dma_start(
                out=xt0[:], in_=xt_d[0].rearrange("(k p) b -> p k b", p=128)))

            late = []
            late += wdma("wzh")
            late += wdma("whh")
            late += wdma("wrx")
            late += wdma("wrh")
            late.append(nc.sync.dma_start(out=b_sb["br"][:], in_=b_d["br"][:]))
            for li in late:
                for pi in (pri[3], pri[7], pri[-1]):  # last wzx/whx chunks + xt0
                    add_dep_helper(li.ins, pi.ins, sync=True,
                                   reason="startup DMA priority")

            def wtile(wname, k, mi):
                off = (k * MT + mi) * 128
                return w_sb[wname][:, off:off + 128]

            h_prev = [None] * MT

            for t in range(T):
                if t == 0:
                    xt_t = xt0
                else:
                    xt_t = xsb.tile([128, KT, NB], mdt, name=f"x{t}", tag="xt")
                    nc.sync.dma_start(
                        out=xt_t[:],
                        in_=xt_d[t].rearrange("(k p) b -> p k b", p=128),
                    )

                def xts(k):
                    return xt_t[:, k, :]

                # --- PSUM banks: z01,z23,r01,r23,g01,g23 (2 m-tiles per bank)
                zb = [psum.tile([128, 512], f32, name=f"z{t}_{i}", tag=f"zb{i}") for i in range(2)]
                gb = [psum.tile([128, 512], f32, name=f"g{t}_{i}", tag=f"gb{i}") for i in range(2)]
                if t > 0:
                    rb = [psum.tile([128, 512], f32, name=f"r{t}_{i}", tag=f"rb{i}") for i in range(2)]

                def half(banks, mi):
                    return banks[mi // 2][:, (mi % 2) * NB:(mi % 2 + 1) * NB]

                # PSUM accumulation groups are tracked per BANK (zero
                # region): exactly one start=True (first MM into the bank)
                # and one stop=True (last MM into the bank) even though the
                # two m-halves are separate output regions.

                # --- x-projections (no recurrence dependency)
                for mi in range(MT):
                    for k in range(KT):
                        nc.tensor.matmul(half(zb, mi), wtile("wzx", k, mi), xts(k),
                                         start=(mi % 2 == 0 and k == 0),
                                         stop=(t == 0 and mi % 2 == 1 and k == KT - 1))
                if t > 0:
                    for mi in range(MT):
                        for k in range(KT):
                            nc.tensor.matmul(half(rb, mi), wtile("wrx", k, mi), xts(k),
                                             start=(mi % 2 == 0 and k == 0), stop=False)
                for mi in range(MT):
                    for k in range(KT):
                        nc.tensor.matmul(half(gb, mi), wtile("whx", k, mi), xts(k),
                                         start=(mi % 2 == 0 and k == 0),
                                         stop=(t == 0 and mi % 2 == 1 and k == KT - 1))

                # --- recurrent parts
                if t > 0:
                    for mi in range(MT):
                        for k in range(KT):
                            nc.tensor.matmul(half(zb, mi), wtile("wzh", k, mi), h_prev[k][:],
                                             start=False,
                                             stop=(mi % 2 == 1 and k == KT - 1))
                    for mi in range(MT):
                        for k in range(KT):
                            nc.tensor.matmul(half(rb, mi), wtile("wrh", k, mi), h_prev[k][:],
                                             start=False,
                                             stop=(mi % 2 == 1 and k == KT - 1))

                    # r gate first (feeds r*h -> Whh matmuls)
                    r_t, rh_t = [], []
                    for mi in range(MT):
                        r_m = ssb.tile([128, NB], f32, name=f"r{t}m{mi}", tag=f"r{mi}")
                        nc.scalar.activation(r_m[:], half(rb, mi), Sig, bias=b_sb["br"][:, mi:mi + 1])
                        r_t.append(r_m)
                    for mi in range(MT):
                        rh_m = ssb.tile([128, NB], mdt, name=f"rh{t}m{mi}", tag=f"rh{mi}")
                        nc.vector.tensor_mul(rh_m[:], r_t[mi][:], h_prev[mi][:].bitcast(f32))
                        rh_t.append(rh_m)

                    for mi in range(MT):
                        for k in range(KT):
                            nc.tensor.matmul(half(gb, mi), wtile("whh", k, mi), rh_t[k][:],
                                             start=False,
                                             stop=(mi % 2 == 1 and k == KT - 1))

                # --- gates and blend
                z_t, g_t, h_t = [], [], []
                for mi in range(MT):
                    z_m = ssb.tile([128, NB], f32, name=f"z{t}m{mi}", tag=f"z{mi}")
                    nc.scalar.activation(z_m[:], half(zb, mi), Sig, bias=b_sb["bz"][:, mi:mi + 1])
                    z_t.append(z_m)
                for mi in range(MT):
                    g_m = ssb.tile([128, NB], f32, name=f"g{t}m{mi}", tag=f"g{mi}")
                    nc.scalar.activation(g_m[:], half(gb, mi), Tanh, bias=b_sb["bh"][:, mi:mi + 1])
                    g_t.append(g_m)

                for mi in range(MT):
                    h_m = hsb.tile([128, NB], mdt, name=f"h{t}m{mi}", tag=f"h{mi}")
                    tmp = ssb.tile([128, NB], f32, name=f"tmp{t}m{mi}", tag=f"tmp{mi}")
                    if t == 0:
                        # h = (1 - z) * g = g - z*g
                        nc.vector.tensor_mul(tmp[:], z_t[mi][:], g_t[mi][:])
                        nc.vector.tensor_sub(h_m[:], g_t[mi][:], tmp[:])
                    else:
                        # h = g + z*(h_prev - g)
                        nc.vector.tensor_sub(tmp[:], h_prev[mi][:].bitcast(f32), g_t[mi][:])
                        nc.vector.tensor_mul(tmp[:], tmp[:], z_t[mi][:])
                        nc.vector.tensor_add(h_m[:], g_t[mi][:], tmp[:])
                    h_t.append(h_m)
                    nc.sync.dma_start(
                        out=out_d[t, mi * 128:(mi + 1) * 128, :], in_=h_m[:].bitcast(f32)
                    )
                h_prev = h_t

    nc.compile()
    return nc


def _get_nc():
    key = MM_DT
    if key not in _CACHE:
        _CACHE[key] = _build_nc()
    return _CACHE[key]


def _pack_w(W):
    # W [dout, din] -> lhsT tiles packed [128, KT*MT*128], tile (k,m) at
    # free offset (k*MT+m)*128: w[p, off+q] = W[m*128+q, k*128+p]
    Wt = np.asarray(W, np.float32).T.reshape(KT, 128, MT, 128)
    return np.ascontiguousarray(Wt.transpose(1, 0, 2, 3).reshape(128, KT * MT * 128))


def kernel(inputss, Wzx, Wzh, Wrx, Wrh, Whx, Whh, bz, br, bh):
    global LAST_RESULT
    from concourse.bass_utils import run_bass_kernel_spmd

    inputss = np.asarray(inputss, np.float32)
    assert inputss.shape == (N, T, D), inputss.shape

    # host-side shard + layout prep
    xs = inputss.reshape(NC, NB, T, D).transpose(0, 2, 3, 1)   # [NC, T, D, NB]
    wp = {"wzx": _pack_w(Wzx), "wzh": _pack_w(Wzh),
          "wrx": _pack_w(Wrx), "wrh": _pack_w(Wrh),
          "whx": _pack_w(Whx), "whh": _pack_w(Whh)}
    bp = {"bz": np.ascontiguousarray(np.asarray(bz, np.float32).reshape(MT, 128).T),
          "br": np.ascontiguousarray(np.asarray(br, np.float32).reshape(MT, 128).T),
          "bh": np.ascontiguousarray(np.asarray(bh, np.float32).reshape(MT, 128).T)}

    in_maps = []
    for c in range(NC):
        m = {"xt": np.ascontiguousarray(xs[c])}
        m.update(wp)
        m.update(bp)
        in_maps.append(m)

    nc = _get_nc()
    trace = bool(int(os.environ.get("GRU_TRACE", "0")))
    res = run_bass_kernel_spmd(nc, in_maps, core_ids=list(range(NC)), trace=trace)
    LAST_RESULT = res

    outs = np.stack([res.results[c]["out"] for c in range(NC)])  # [NC, T, D, NB]
    return np.ascontiguousarray(outs.transpose(0, 3, 1, 2).reshape(N, T, D))



# revision 3
# speedup vs baseline: 1.2024x; 1.2024x over previous
"""GRU layer kernel for Trainium2, data-parallel over 8 NeuronCores.

Strategy (feature-major / weight-stationary, fp16 matmul path):
  - Shard batch N=2048 -> 8 cores x NB=256.
  - On host: transpose inputs to feature-major xT[t] = [D, NB] (fp16) per
    core, and pre-pack each weight matrix W[dout,din] into the PE lhsT tile
    layout (tile (k,m): lhsT[p, q] = W[m*128+q, k*128+p]) in fp16.
  - On chip per timestep (matmuls fp16 @ 1 cyc/row full rate, PSUM fp32):
      z_pre.T = Wz_x| x_t.T  +  Wz_h| h.T      (8 K-tiles into PSUM)
      r_pre.T = likewise
      g_pre.T = Wh_x| x_t.T  +  Wh_h| (r*h).T
      z,r = sigmoid on ACT; g = tanh; blend on DVE — all fp16 tiles so the
      DVE runs in 2x mode. h state is fp16 (sim rel-err 9.1e-4 vs fp32 ref).
    x-projection matmuls of step t+1 have no dependency on step t and fill
    the PE pipeline while ACT/DVE resolve the recurrence.
  - Output written feature-major fp16 [T, D, NB] per core; host transposes
    back and casts to fp32.
"""
import os
import numpy as np

N, D = 2048, 512
T = int(os.environ.get("GRU_T", "64"))
NC = 8
NB = N // NC          # 256 batch rows per core
KT = D // 128         # 4 k-tiles
MT = D // 128         # 4 m-tiles

MM_DT = os.environ.get("GRU_MM_DT", "fp16")   # fp16 | bf16 | fp32r

_CACHE = {}
LAST_RESULT = None


def _build_nc(zero_bias):
    import concourse.bacc as bacc
    import concourse.mybir as mybir
    from concourse.tile import TileContext

    f32 = mybir.dt.float32
    mdt = {"fp16": mybir.dt.float16, "bf16": mybir.dt.bfloat16,
           "fp32r": mybir.dt.float32r}[MM_DT]
    Sig = mybir.ActivationFunctionType.Sigmoid
    Tanh = mybir.ActivationFunctionType.Tanh

    nc = bacc.Bacc("TRN2", target_bir_lowering=False, debug=False, num_devices=NC)

    xt_d = nc.dram_tensor("xt", [T, D, NB], mdt, kind="ExternalInput")
    w_d = {}
    for wname in ("wzx", "wzh", "wrx", "wrh", "whx", "whh"):
        w_d[wname] = nc.dram_tensor(wname, [128, KT * MT * 128], mdt, kind="ExternalInput")
    b_d = {}
    for bname in ("bz", "br", "bh"):
        b_d[bname] = nc.dram_tensor(bname, [128, MT], f32, kind="ExternalInput")
    out_d = nc.dram_tensor("out", [T, D, NB], mdt, kind="ExternalOutput")

    with TileContext(nc) as tc:
        with (
            tc.tile_pool(name="wsb", bufs=1) as wsb,
            tc.tile_pool(name="xsb", bufs=4) as xsb,
            tc.tile_pool(name="ssb", bufs=2) as ssb,
            tc.tile_pool(name="hsb", bufs=3) as hsb,
            tc.tile_pool(name="psum", bufs=1, space="PSUM") as psum,
        ):
            w_sb = {}
            for wname in w_d:
                w_sb[wname] = wsb.tile([128, KT * MT * 128], mdt, name=f"w_{wname}")
            b_sb = {}
            for bname in b_d:
                b_sb[bname] = wsb.tile([128, MT], f32, name=f"b_{bname}")

            from concourse.tile import add_dep_helper

            def wdma(wname, nchunks=4):
                # chunked so the pieces spread across DMA queues
                insts = []
                cw = KT * MT * 128 // nchunks
                for u in range(nchunks):
                    insts.append(
                        nc.sync.dma_start(out=w_sb[wname][:, u * cw:(u + 1) * cw],
                                          in_=w_d[wname][:, u * cw:(u + 1) * cw]))
                return insts

            # priority set: everything t=0's first matmuls need. All DMA
            # queues round-robin, so the late weight DMAs are gated on the
            # priority set's completion to give it the full HBM bandwidth.
            pri = []
            pri += wdma("wzx")
            pri += wdma("whx")
            pri.append(nc.sync.dma_start(out=b_sb["bz"][:], in_=b_d["bz"][:]))
            pri.append(nc.sync.dma_start(out=b_sb["bh"][:], in_=b_d["bh"][:]))
            xt0 = xsb.tile([128, KT, NB], mdt, name="x0", tag="xt")
            pri.append(nc.sync.dma_start(
                out=xt0[:], in_=xt_d[0].rearrange("(k p) b -> p k b", p=128)))

            late = []
            late += wdma("wzh")
            late += wdma("whh")
            late += wdma("wrx")
            late += wdma("wrh")
            late.append(nc.sync.dma_start(out=b_sb["br"][:], in_=b_d["br"][:]))
            for li in late:
                for pi in (pri[3], pri[7], pri[-1]):  # last wzx/whx chunks + xt0
                    add_dep_helper(li.ins, pi.ins, sync=True,
                                   reason="startup DMA priority")

            def wtile(wname, k, mi):
                off = (k * MT + mi) * 128
                return w_sb[wname][:, off:off + 128]

            h_prev = [None] * MT

            for t in range(T):
                if t == 0:
                    xt_t = xt0
                else:
                    xt_t = xsb.tile([128, KT, NB], mdt, name=f"x{t}", tag="xt")
                    nc.sync.dma_start(
                        out=xt_t[:],
                        in_=xt_d[t].rearrange("(k p) b -> p k b", p=128),
                    )

                def xts(k):
                    return xt_t[:, k, :]

                # --- PSUM banks: z01,z23,r01,r23,g01,g23 (2 m-tiles per bank)
                zb = [psum.tile([128, 512], f32, name=f"z{t}_{i}", tag=f"zb{i}") for i in range(2)]
                gb = [psum.tile([128, 512], f32, name=f"g{t}_{i}", tag=f"gb{i}") for i in range(2)]
                if t > 0:
                    rb = [psum.tile([128, 512], f32, name=f"r{t}_{i}", tag=f"rb{i}") for i in range(2)]

                def half(banks, mi):
                    return banks[mi // 2][:, (mi % 2) * NB:(mi % 2 + 1) * NB]

                # PSUM accumulation groups are tracked per BANK (zero
                # region): exactly one start=True (first MM into the bank)
                # and one stop=True (last MM into the bank) even though the
                # two m-halves are separate output regions.

                # --- x-projections (no recurrence dependency)
                for mi in range(MT):
                    for k in range(KT):
                        nc.tensor.matmul(half(zb, mi), wtile("wzx", k, mi), xts(k),
                                         start=(mi % 2 == 0 and k == 0),
                                         stop=(t == 0 and mi % 2 == 1 and k == KT - 1))
                if t > 0:
                    for mi in range(MT):
                        for k in range(KT):
                            nc.tensor.matmul(half(rb, mi), wtile("wrx", k, mi), xts(k),
                                             start=(mi % 2 == 0 and k == 0), stop=False)
                for mi in range(MT):
                    for k in range(KT):
                        nc.tensor.matmul(half(gb, mi), wtile("whx", k, mi), xts(k),
                                         start=(mi % 2 == 0 and k == 0),
                                         stop=(t == 0 and mi % 2 == 1 and k == KT - 1))

                # --- recurrent parts
                if t > 0:
                    def hview(k):
                        return h_prev[k // 2][:, k % 2, :]

                    for mi in range(MT):
                        for k in range(KT):
                            nc.tensor.matmul(half(zb, mi), wtile("wzh", k, mi), hview(k),
                                             start=False,
                                             stop=(mi % 2 == 1 and k == KT - 1))
                    for mi in range(MT):
                        for k in range(KT):
                            nc.tensor.matmul(half(rb, mi), wtile("wrh", k, mi), hview(k),
                                             start=False,
                                             stop=(mi % 2 == 1 and k == KT - 1))

                    # r gate first (feeds r*h -> Whh matmuls)
                    r_t, rh_t = [], []
                    for bi in range(2):
                        r_m = ssb.tile([128, 2, NB], mdt, name=f"r{t}b{bi}", tag=f"r{bi}")
                        if zero_bias:
                            nc.scalar.activation(
                                r_m[:], rb[bi][:].rearrange("p (m b) -> p m b", m=2), Sig)
                        else:
                            for j in range(2):
                                nc.scalar.activation(
                                    r_m[:, j, :], half(rb, 2 * bi + j), Sig,
                                    bias=b_sb["br"][:, 2 * bi + j:2 * bi + j + 1])
                        r_t.append(r_m)
                    for bi in range(2):
                        rh_m = ssb.tile([128, 2, NB], mdt, name=f"rh{t}b{bi}", tag=f"rh{bi}")
                        nc.vector.tensor_mul(rh_m[:], r_t[bi][:], h_prev[bi][:])
                        rh_t.append(rh_m)

                    def rhs_rh(k):
                        return rh_t[k // 2][:, k % 2, :]

                    for mi in range(MT):
                        for k in range(KT):
                            nc.tensor.matmul(half(gb, mi), wtile("whh", k, mi), rhs_rh(k),
                                             start=False,
                                             stop=(mi % 2 == 1 and k == KT - 1))

                # --- gates and blend
                z_t, g_t = [], []
                for bi in range(2):
                    z_m = ssb.tile([128, 2, NB], mdt, name=f"z{t}b{bi}", tag=f"z{bi}")
                    if zero_bias:
                        nc.scalar.activation(
                            z_m[:], zb[bi][:].rearrange("p (m b) -> p m b", m=2), Sig)
                    else:
                        for j in range(2):
                            nc.scalar.activation(
                                z_m[:, j, :], half(zb, 2 * bi + j), Sig,
                                bias=b_sb["bz"][:, 2 * bi + j:2 * bi + j + 1])
                    z_t.append(z_m)
                for bi in range(2):
                    g_m = ssb.tile([128, 2, NB], mdt, name=f"g{t}b{bi}", tag=f"g{bi}")
                    if zero_bias:
                        nc.scalar.activation(
                            g_m[:], gb[bi][:].rearrange("p (m b) -> p m b", m=2), Tanh)
                    else:
                        for j in range(2):
                            nc.scalar.activation(
                                g_m[:, j, :], half(gb, 2 * bi + j), Tanh,
                                bias=b_sb["bh"][:, 2 * bi + j:2 * bi + j + 1])
                    g_t.append(g_m)

                h_t = []
                for bi in range(2):
                    hp = hsb.tile([128, 2, NB], mdt, name=f"h{t}p{bi}", tag=f"hp{bi}")
                    tmp = ssb.tile([128, 2, NB], mdt, name=f"tmp{t}b{bi}", tag=f"tmp{bi}")
                    if t == 0:
                        # h = (1 - z) * g = g - z*g
                        nc.vector.tensor_mul(tmp[:], z_t[bi][:], g_t[bi][:])
                        nc.vector.tensor_sub(hp[:], g_t[bi][:], tmp[:])
                    else:
                        # h = g + z*(h_prev - g)
                        nc.vector.tensor_sub(tmp[:], h_prev[bi][:], g_t[bi][:])
                        nc.vector.tensor_mul(tmp[:], tmp[:], z_t[bi][:])
                        nc.vector.tensor_add(hp[:], g_t[bi][:], tmp[:])
                    h_t.append(hp)
                    nc.sync.dma_start(
                        out=out_d[t, 2 * bi * 128:(2 * bi + 2) * 128, :]
                        .rearrange("(m p) b -> p m b", p=128),
                        in_=hp[:])
                h_prev = h_t

    nc.compile()
    return nc


def _get_nc(zero_bias):
    key = (MM_DT, zero_bias)
    if key not in _CACHE:
        _CACHE[key] = _build_nc(zero_bias)
    return _CACHE[key]


def _np_mdt():
    import ml_dtypes
    return {"fp16": np.float16, "bf16": ml_dtypes.bfloat16,
            "fp32r": np.float32}[MM_DT]


def _pack_w(W):
    # W [dout, din] -> lhsT tiles packed [128, KT*MT*128], tile (k,m) at
    # free offset (k*MT+m)*128: w[p, off+q] = W[m*128+q, k*128+p]
    Wt = np.asarray(W, np.float32).T.reshape(KT, 128, MT, 128)
    return np.ascontiguousarray(
        Wt.transpose(1, 0, 2, 3).reshape(128, KT * MT * 128)).astype(_np_mdt())


def kernel(inputss, Wzx, Wzh, Wrx, Wrh, Whx, Whh, bz, br, bh):
    global LAST_RESULT
    from concourse.bass_utils import run_bass_kernel_spmd

    inputss = np.asarray(inputss, np.float32)
    assert inputss.shape == (N, T, D), inputss.shape

    zero_bias = (not np.any(np.asarray(bz)) and not np.any(np.asarray(br))
                 and not np.any(np.asarray(bh)))

    # host-side shard + layout prep
    xs = inputss.reshape(NC, NB, T, D).transpose(0, 2, 3, 1)   # [NC, T, D, NB]
    xs = xs.astype(_np_mdt())
    wp = {"wzx": _pack_w(Wzx), "wzh": _pack_w(Wzh),
          "wrx": _pack_w(Wrx), "wrh": _pack_w(Wrh),
          "whx": _pack_w(Whx), "whh": _pack_w(Whh)}
    bp = {"bz": np.ascontiguousarray(np.asarray(bz, np.float32).reshape(MT, 128).T),
          "br": np.ascontiguousarray(np.asarray(br, np.float32).reshape(MT, 128).T),
          "bh": np.ascontiguousarray(np.asarray(bh, np.float32).reshape(MT, 128).T)}

    in_maps = []
    for c in range(NC):
        m = {"xt": np.ascontiguousarray(xs[c])}
        m.update(wp)
        m.update(bp)
        in_maps.append(m)

    nc = _get_nc(zero_bias)
    trace = bool(int(os.environ.get("GRU_TRACE", "0")))
    res = run_bass_kernel_spmd(nc, in_maps, core_ids=list(range(NC)), trace=trace)
    LAST_RESULT = res

    outs = np.stack([np.asarray(res.results[c]["out"]) for c in range(NC)])  # [NC, T, D, NB]
    return np.ascontiguousarray(
        outs.astype(np.float32).transpose(0, 3, 1, 2).reshape(N, T, D))


# revision 14
# speedup vs baseline: 1.4433x; 1.2003x over previous
"""GRU layer kernel for Trainium2, data-parallel over 8 NeuronCores.

Strategy (feature-major / weight-stationary, fp16 matmul path):
  - Shard batch N=2048 -> 8 cores x NB=256.
  - On host: transpose inputs to feature-major xT[t] = [D, NB] (fp16) per
    core, and pre-pack each weight matrix W[dout,din] into the PE lhsT tile
    layout (tile (k,m): lhsT[p, q] = W[m*128+q, k*128+p]) in fp16.
  - On chip per timestep (matmuls fp16 @ 1 cyc/row full rate, PSUM fp32):
      z_pre.T = Wz_x| x_t.T  +  Wz_h| h.T      (8 K-tiles into PSUM)
      r_pre.T = likewise
      g_pre.T = Wh_x| x_t.T  +  Wh_h| (r*h).T
      z,r = sigmoid on ACT; g = tanh; blend on DVE — all fp16 tiles so the
      DVE runs in 2x mode. h state is fp16 (sim rel-err 9.1e-4 vs fp32 ref).
    x-projection matmuls of step t+1 have no dependency on step t and fill
    the PE pipeline while ACT/DVE resolve the recurrence.
  - Output written feature-major fp16 [T, D, NB] per core; host transposes
    back and casts to fp32.
"""
import os
import numpy as np

N, D = 2048, 512
T = int(os.environ.get("GRU_T", "64"))
NC = 8
NB = N // NC          # 256 batch rows per core
KT = D // 128         # 4 k-tiles
MT = D // 128         # 4 m-tiles

MM_DT = os.environ.get("GRU_MM_DT", "fp16")   # fp16 | bf16 | fp32r
R8 = bool(int(os.environ.get("GRU_R8", "1")))  # r-gate matmuls in fp8 DoubleRow

# fp8 scaling: psum_r = (W*SWX)@(x*SX) + (W*SWH)@(h*SH), both products 2^16,
# descaled inside the sigmoid via activation scale=2^-16.
SX, SH = 16.0, 128.0
SWX, SWH = 4096.0, 512.0
PROD = SWX * SX  # == SWH * SH == 65536

_CACHE = {}
LAST_RESULT = None


def _build_nc(zero_bias):
    import concourse.bacc as bacc
    import concourse.mybir as mybir
    from concourse.tile import TileContext

    f32 = mybir.dt.float32
    f8 = mybir.dt.float8e4
    DR = mybir.MatmulPerfMode.DoubleRow
    mdt = {"fp16": mybir.dt.float16, "bf16": mybir.dt.bfloat16,
           "fp32r": mybir.dt.float32r}[MM_DT]
    Sig = mybir.ActivationFunctionType.Sigmoid
    Tanh = mybir.ActivationFunctionType.Tanh
    Copy = mybir.ActivationFunctionType.Copy

    nc = bacc.Bacc("TRN2", target_bir_lowering=False, debug=False, num_devices=NC)

    xt_d = nc.dram_tensor("xt", [T, D, NB], mdt, kind="ExternalInput")
    wnames = ["wzx", "wzh", "whx", "whh"] + ([] if R8 else ["wrx", "wrh"])
    w_d = {}
    for wname in wnames:
        w_d[wname] = nc.dram_tensor(wname, [128, KT * MT * 128], mdt, kind="ExternalInput")
    if R8:
        # DoubleRow-packed fp8 r-gate weights: [128, (k2 mi i m)] with
        # w[p, k2, mi, i, m] = W[mi*128+m, (2*k2+i)*128+p] * scale
        xt8_d = nc.dram_tensor("xt8", [T, D, NB], f8, kind="ExternalInput")
        w8_d = {w: nc.dram_tensor(w, [128, 2 * MT * 2 * 128], f8, kind="ExternalInput")
                for w in ("wrx8", "wrh8")}
    b_d = {}
    for bname in ("bz", "br", "bh"):
        b_d[bname] = nc.dram_tensor(bname, [128, MT], f32, kind="ExternalInput")
    out_d = nc.dram_tensor("out", [T, D, NB], mdt, kind="ExternalOutput")

    with TileContext(nc) as tc:
        with (
            tc.tile_pool(name="wsb", bufs=1) as wsb,
            tc.tile_pool(name="xsb", bufs=4) as xsb,
            tc.tile_pool(name="ssb", bufs=2) as ssb,
            tc.tile_pool(name="hsb", bufs=3) as hsb,
            tc.tile_pool(name="psum", bufs=1, space="PSUM") as psum,
        ):
            w_sb = {}
            for wname in w_d:
                w_sb[wname] = wsb.tile([128, KT * MT * 128], mdt, name=f"w_{wname}")
            if R8:
                w8_sb = {w: wsb.tile([128, 2 * MT, 2, 128], f8, name=f"w_{w}")
                         for w in ("wrx8", "wrh8")}
            b_sb = {}
            for bname in b_d:
                b_sb[bname] = wsb.tile([128, MT], f32, name=f"b_{bname}")

            from concourse.tile import add_dep_helper

            def wdma(wname, nchunks=4):
                # chunked so the pieces spread across DMA queues
                insts = []
                cw = KT * MT * 128 // nchunks
                for u in range(nchunks):
                    insts.append(
                        nc.sync.dma_start(out=w_sb[wname][:, u * cw:(u + 1) * cw],
                                          in_=w_d[wname][:, u * cw:(u + 1) * cw]))
                return insts

            # priority set: everything t=0's first matmuls need. All DMA
            # queues round-robin, so the late weight DMAs are gated on the
            # priority set's completion to give it the full HBM bandwidth.
            pri = []
            pri += wdma("wzx")
            pri += wdma("whx")
            pri.append(nc.sync.dma_start(out=b_sb["bz"][:], in_=b_d["bz"][:]))
            pri.append(nc.sync.dma_start(out=b_sb["bh"][:], in_=b_d["bh"][:]))
            xt0 = xsb.tile([128, KT, NB], mdt, name="x0", tag="xt")
            pri.append(nc.sync.dma_start(
                out=xt0[:], in_=xt_d[0].rearrange("(k p) b -> p k b", p=128)))

            late = []
            late += wdma("wzh")
            late += wdma("whh")
            if R8:
                for w in ("wrx8", "wrh8"):
                    late.append(nc.sync.dma_start(
                        out=w8_sb[w][:],
                        in_=w8_d[w][:].rearrange("p (a i m) -> p a i m", a=2 * MT, i=2)))
            else:
                late += wdma("wrx")
                late += wdma("wrh")
            late.append(nc.sync.dma_start(out=b_sb["br"][:], in_=b_d["br"][:]))
            for li in late:
                for pi in (pri[3], pri[7], pri[-1]):  # last wzx/whx chunks + xt0
                    add_dep_helper(li.ins, pi.ins, sync=True,
                                   reason="startup DMA priority")

            def wtile(wname, k, mi):
                off = (k * MT + mi) * 128
                return w_sb[wname][:, off:off + 128]

            h_prev = [None] * MT

            for t in range(T):
                if t == 0:
                    xt_t = xt0
                else:
                    xt_t = xsb.tile([128, KT, NB], mdt, name=f"x{t}", tag="xt")
                    nc.sync.dma_start(
                        out=xt_t[:],
                        in_=xt_d[t].rearrange("(k p) b -> p k b", p=128),
                    )
                    if R8:
                        xt8_t = xsb.tile([128, 2, 2, NB], f8, name=f"x8{t}", tag="xt8")
                        nc.sync.dma_start(
                            out=xt8_t[:],
                            in_=xt8_d[t].rearrange("(k i p) b -> p k i b", k=2, i=2),
                        )

                def xts(k):
                    return xt_t[:, k, :]

                # --- PSUM banks: z01,z23,r01,r23,g01,g23 (2 m-tiles per bank)
                zb = [psum.tile([128, 512], f32, name=f"z{t}_{i}", tag=f"zb{i}") for i in range(2)]
                gb = [psum.tile([128, 512], f32, name=f"g{t}_{i}", tag=f"gb{i}") for i in range(2)]
                if t > 0:
                    rb = [psum.tile([128, 512], f32, name=f"r{t}_{i}", tag=f"rb{i}") for i in range(2)]

                def half(banks, mi):
                    return banks[mi // 2][:, (mi % 2) * NB:(mi % 2 + 1) * NB]

                # PSUM accumulation groups are tracked per BANK (zero
                # region): exactly one start=True (first MM into the bank)
                # and one stop=True (last MM into the bank) even though the
                # two m-halves are separate output regions.

                # --- x-projections (no recurrence dependency)
                for mi in range(MT):
                    for k in range(KT):
                        nc.tensor.matmul(half(zb, mi), wtile("wzx", k, mi), xts(k),
                                         start=(mi % 2 == 0 and k == 0),
                                         stop=(t == 0 and mi % 2 == 1 and k == KT - 1))
                if t > 0:
                    if R8:
                        for mi in range(MT):
                            for k2 in range(2):
                                nc.tensor.matmul(half(rb, mi),
                                                 w8_sb["wrx8"][:, k2 * MT + mi, :, :],
                                                 xt8_t[:, k2],
                                                 start=(mi % 2 == 0 and k2 == 0),
                                                 stop=False, perf_mode=DR)
                    else:
                        for mi in range(MT):
                            for k in range(KT):
                                nc.tensor.matmul(half(rb, mi), wtile("wrx", k, mi), xts(k),
                                                 start=(mi % 2 == 0 and k == 0), stop=False)
                for mi in range(MT):
                    for k in range(KT):
                        nc.tensor.matmul(half(gb, mi), wtile("whx", k, mi), xts(k),
                                         start=(mi % 2 == 0 and k == 0),
                                         stop=(t == 0 and mi % 2 == 1 and k == KT - 1))

                # --- recurrent parts
                if t > 0:
                    def hview(k):
                        return h_prev[k // 2][:, k % 2, :]

                    for mi in range(MT):
                        for k in range(KT):
                            nc.tensor.matmul(half(zb, mi), wtile("wzh", k, mi), hview(k),
                                             start=False,
                                             stop=(mi % 2 == 1 and k == KT - 1))
                    if R8:
                        for mi in range(MT):
                            for k2 in range(2):
                                nc.tensor.matmul(half(rb, mi),
                                                 w8_sb["wrh8"][:, k2 * MT + mi, :, :],
                                                 h8_prev[k2][:],
                                                 start=False,
                                                 stop=(mi % 2 == 1 and k2 == 1),
                                                 perf_mode=DR)
                    else:
                        for mi in range(MT):
                            for k in range(KT):
                                nc.tensor.matmul(half(rb, mi), wtile("wrh", k, mi), hview(k),
                                                 start=False,
                                                 stop=(mi % 2 == 1 and k == KT - 1))

                    # r gate first (feeds r*h -> Whh matmuls)
                    rsc = (1.0 / PROD) if R8 else 1.0
                    r_t, rh_t = [], []
                    for bi in range(2):
                        r_m = ssb.tile([128, 2, NB], mdt, name=f"r{t}b{bi}", tag=f"r{bi}")
                        if zero_bias:
                            nc.scalar.activation(
                                r_m[:], rb[bi][:].rearrange("p (m b) -> p m b", m=2), Sig,
                                scale=rsc)
                        else:
                            for j in range(2):
                                nc.scalar.activation(
                                    r_m[:, j, :], half(rb, 2 * bi + j), Sig,
                                    bias=b_sb["br"][:, 2 * bi + j:2 * bi + j + 1],
                                    scale=rsc)
                        r_t.append(r_m)
                    for bi in range(2):
                        rh_m = ssb.tile([128, 2, NB], mdt, name=f"rh{t}b{bi}", tag=f"rh{bi}")
                        nc.vector.tensor_mul(rh_m[:], r_t[bi][:], h_prev[bi][:])
                        rh_t.append(rh_m)

                    def rhs_rh(k):
                        return rh_t[k // 2][:, k % 2, :]

                    for mi in range(MT):
                        for k in range(KT):
                            nc.tensor.matmul(half(gb, mi), wtile("whh", k, mi), rhs_rh(k),
                                             start=False,
                                             stop=(mi % 2 == 1 and k == KT - 1))

                # --- gates and blend
                z_t, g_t = [], []
                for bi in range(2):
                    z_m = ssb.tile([128, 2, NB], mdt, name=f"z{t}b{bi}", tag=f"z{bi}")
                    if zero_bias:
                        nc.scalar.activation(
                            z_m[:], zb[bi][:].rearrange("p (m b) -> p m b", m=2), Sig)
                    else:
                        for j in range(2):
                            nc.scalar.activation(
                                z_m[:, j, :], half(zb, 2 * bi + j), Sig,
                                bias=b_sb["bz"][:, 2 * bi + j:2 * bi + j + 1])
                    z_t.append(z_m)
                for bi in range(2):
                    g_m = ssb.tile([128, 2, NB], mdt, name=f"g{t}b{bi}", tag=f"g{bi}")
                    if zero_bias:
                        nc.scalar.activation(
                            g_m[:], gb[bi][:].rearrange("p (m b) -> p m b", m=2), Tanh)
                    else:
                        for j in range(2):
                            nc.scalar.activation(
                                g_m[:, j, :], half(gb, 2 * bi + j), Tanh,
                                bias=b_sb["bh"][:, 2 * bi + j:2 * bi + j + 1])
                    g_t.append(g_m)

                h_t = []
                for bi in range(2):
                    hp = hsb.tile([128, 2, NB], mdt, name=f"h{t}p{bi}", tag=f"hp{bi}")
                    tmp = ssb.tile([128, 2, NB], mdt, name=f"tmp{t}b{bi}", tag=f"tmp{bi}")
                    if t == 0:
                        # h = (1 - z) * g = g - z*g
                        nc.vector.tensor_mul(tmp[:], z_t[bi][:], g_t[bi][:])
                        nc.vector.tensor_sub(hp[:], g_t[bi][:], tmp[:])
                    else:
                        # h = g + z*(h_prev - g)
                        nc.vector.tensor_sub(tmp[:], h_prev[bi][:], g_t[bi][:])
                        nc.vector.tensor_mul(tmp[:], tmp[:], z_t[bi][:])
                        nc.vector.tensor_add(hp[:], g_t[bi][:], tmp[:])
                    h_t.append(hp)
                    nc.sync.dma_start(
                        out=out_d[t, 2 * bi * 128:(2 * bi + 2) * 128, :]
                        .rearrange("(m p) b -> p m b", p=128),
                        in_=hp[:])
                h_prev = h_t
                if R8 and t < T - 1:
                    h8_prev = []
                    for bi in range(2):
                        h8 = ssb.tile([128, 2, NB], f8, name=f"h8{t}b{bi}", tag=f"h8{bi}")
                        nc.scalar.activation(h8[:], h_t[bi][:], Copy, scale=SH)
                        h8_prev.append(h8)

    nc.compile()
    return nc


def _get_nc(zero_bias):
    key = (MM_DT, zero_bias, R8)
    if key not in _CACHE:
        _CACHE[key] = _build_nc(zero_bias)
    return _CACHE[key]


def _np_mdt():
    import ml_dtypes
    return {"fp16": np.float16, "bf16": ml_dtypes.bfloat16,
            "fp32r": np.float32}[MM_DT]


def _pack_w(W):
    # W [dout, din] -> lhsT tiles packed [128, KT*MT*128], tile (k,m) at
    # free offset (k*MT+m)*128: w[p, off+q] = W[m*128+q, k*128+p]
    Wt = np.asarray(W, np.float32).T.reshape(KT, 128, MT, 128)
    return np.ascontiguousarray(
        Wt.transpose(1, 0, 2, 3).reshape(128, KT * MT * 128)).astype(_np_mdt())


def _q8(a, scale):
    import ml_dtypes
    return np.clip(np.asarray(a, np.float32) * scale, -240, 240).astype(
        ml_dtypes.float8_e4m3)


def _pack_w8(W, scale):
    # DoubleRow lhsT: w8[p, (k2 mi i m)] = W[mi*128+m, (2*k2+i)*128+p] * scale
    # W.T [din, dout] -> [k2, i, p, mi, m] -> [p, k2, mi, i, m]
    Wt = np.asarray(W, np.float32).T.reshape(2, 2, 128, MT, 128)
    Wt = Wt.transpose(2, 0, 3, 1, 4).reshape(128, 2 * MT * 2 * 128)
    return np.ascontiguousarray(_q8(Wt, scale))


def kernel(inputss, Wzx, Wzh, Wrx, Wrh, Whx, Whh, bz, br, bh):
    global LAST_RESULT
    from concourse.bass_utils import run_bass_kernel_spmd

    inputss = np.asarray(inputss, np.float32)
    assert inputss.shape == (N, T, D), inputss.shape

    zero_bias = (not np.any(np.asarray(bz)) and not np.any(np.asarray(br))
                 and not np.any(np.asarray(bh)))

    # host-side shard + layout prep
    xs32 = inputss.reshape(NC, NB, T, D).transpose(0, 2, 3, 1)  # [NC, T, D, NB]
    xs = xs32.astype(_np_mdt())
    wp = {"wzx": _pack_w(Wzx), "wzh": _pack_w(Wzh),
          "whx": _pack_w(Whx), "whh": _pack_w(Whh)}
    if R8:
        wp["wrx8"] = _pack_w8(Wrx, SWX)
        wp["wrh8"] = _pack_w8(Wrh, SWH)
        xs8 = _q8(xs32, SX)
    else:
        wp["wrx"] = _pack_w(Wrx)
        wp["wrh"] = _pack_w(Wrh)
    bp = {"bz": np.ascontiguousarray(np.asarray(bz, np.float32).reshape(MT, 128).T),
          "br": np.ascontiguousarray(np.asarray(br, np.float32).reshape(MT, 128).T),
          "bh": np.ascontiguousarray(np.asarray(bh, np.float32).reshape(MT, 128).T)}

    in_maps = []
    for c in range(NC):
        m = {"xt": np.ascontiguousarray(xs[c])}
        if R8:
            m["xt8"] = np.ascontiguousarray(xs8[c])
        m.update(wp)
        m.update(bp)
        in_maps.append(m)

    nc = _get_nc(zero_bias)
    trace = bool(int(os.environ.get("GRU_TRACE", "0")))
    res = run_bass_kernel_spmd(nc, in_maps, core_ids=list(range(NC)), trace=trace)
    LAST_RESULT = res

    outs = np.stack([np.asarray(res.results[c]["out"]) for c in range(NC)])  # [NC, T, D, NB]
    return np.ascontiguousarray(
        outs.astype(np.float32).transpose(0, 3, 1, 2).reshape(N, T, D))
